# revision 2
# baseline (speedup 1.0000x reference)
"""Trainium2 Bass kernel: CausalParticleAttention (v3: bf16 + fp8-DR S).

B=16, N=16, T=48, C=512, H=8 (hd=64); L = N*T = 768 per batch;
causal over time only; relative time/particle biases; o-projection.
Data parallel: 2 batches/core x 8 cores.

Design (see session notes):
  * t-major rows (i = t*16 + n): causal mask = block-lower-triangular at
    128-col granularity -> S/exp/PV computed only for i >= 128*jc (58%).
  * bf16 data path everywhere precision matters (x, W, P, v, y, Wo):
    fp8 was measured at 6e-2 rel err vs the 2e-2 gate.
  * S^T matmul keeps the fp8e4 DoubleRow trick (0.5 c/row): contraction
    [64 q-dims | 64 aug] as 2 groups on 64 partitions; aug =
    onehot(t1,n1) x (8*bias + mask(-240)); exp scale 1/8. fp8 q/k
    storage costs ~1e-2 err (measured; within gate with margin).
  * Natural-orientation PV (out y[i-rows, 65], ones col 0 = rowsum):
    streams the 65-wide side (1365 c/unit); reciprocal is per-partition
    [128, 6]; normalize = one DVE pass with free-dim broadcast bf16.
  * y -> yT transpose via 6 XBAR DMA transposes per head pair
    ([128,128] bf16, 14 ns/tile; 3D XBAR crashes the device - don't).
  * Projections / o-projection ride inside the sp0/sp12 PSUM rotation;
    software pipeline: exp(n) -> S(n+1) -> rider -> PV(n) so ACT rarely
    starves. PSUM: sp0 3 + sp12 2.5 + yp 2x0.76 = 7 banks.
  * oh/kb/out DMAs issued via SWDGE (gpsimd) to keep the 625ns/issue
    HWDGE device under the XBAR + input load.
"""

import sys

sys.path.insert(0, "/opt/trn_rl_repo")

import numpy as np
import ml_dtypes

import concourse.bacc as bacc
import concourse.bass as bass
import concourse.mybir as mybir
import concourse.tile as tile
from concourse import bass_utils

F32 = mybir.dt.float32
BF16 = mybir.dt.bfloat16
F8 = mybir.dt.float8e4
DR = mybir.MatmulPerfMode.DoubleRow
EXP = mybir.ActivationFunctionType.Exp
MUL = mybir.AluOpType.mult
FP8 = ml_dtypes.float8_e4m3
BF = ml_dtypes.bfloat16

B_TOT, H, T, NP, C = 16, 8, 48, 16, 512
HD = C // H              # 64
L = NP * T               # 768
NCORES = 8
BPC = B_TOT // NCORES    # 2
R = BPC * L              # 1536
MASK = -240.0            # pre-scale; post-scale -30
ESCALE = 0.125           # exp scale = 1/sqrt(hd)

SP0_LEN, SP12_LEN = 1536, 1280
S_OFF = {0: 0, 1: 768, 2: 0, 3: 512, 4: 896, 5: 1152}
S_I0 = {0: 0, 1: 128, 2: 256, 3: 384, 4: 512, 5: 640}


def _bank_pieces(c0, c1):
    out = []
    while c0 < c1:
        nxt = min(c1, (c0 // 512 + 1) * 512)
        out.append((c0, nxt))
        c0 = nxt
    return out


def build_nc():
    nc = bacc.Bacc("TRN2", target_bir_lowering=False, debug=False)
    xt_d = nc.dram_tensor("xt", [C, R], BF16, kind="ExternalInput").ap()
    wq_d = nc.dram_tensor("wq", [C, C], BF16, kind="ExternalInput").ap()
    wk_d = nc.dram_tensor("wk", [C, C], BF16, kind="ExternalInput").ap()
    wv_d = nc.dram_tensor("wv", [C, C], BF16, kind="ExternalInput").ap()
    wo_d = nc.dram_tensor("wo", [C, C], BF16, kind="ExternalInput").ap()
    oh_d = nc.dram_tensor("oh", [128, L], F8, kind="ExternalInput").ap()
    kb_d = nc.dram_tensor("kb", [4, 128, L], F8, kind="ExternalInput").ap()
    out_d = nc.dram_tensor("out", [R, C], F32, kind="ExternalOutput").ap()
    with tile.TileContext(nc) as tc:
        _body(tc, xt_d, wq_d, wk_d, wv_d, wo_d, oh_d, kb_d, out_d)
    nc.compile()
    return nc


def _body(tc, xt_d, wq_d, wk_d, wv_d, wo_d, oh_d, kb_d, out_d):
    nc = tc.nc
    from contextlib import ExitStack

    with ExitStack() as ctx:
        persist = ctx.enter_context(tc.tile_pool(name="persist", bufs=1))
        pts0_pool = ctx.enter_context(tc.tile_pool(name="pts0", bufs=3))
        pts12_pool = ctx.enter_context(tc.tile_pool(name="pts12", bufs=3))
        yn_pool = ctx.enter_context(tc.tile_pool(name="yn", bufs=3))
        rc_pool = ctx.enter_context(tc.tile_pool(name="rc", bufs=4))
        ys_pool = ctx.enter_context(tc.tile_pool(name="ys", bufs=3))
        fo_pool = ctx.enter_context(tc.tile_pool(name="fo", bufs=4))
        sp0_pool = ctx.enter_context(tc.tile_pool(name="sp0", bufs=1, space="PSUM"))
        sp12_pool = ctx.enter_context(tc.tile_pool(name="sp12", bufs=1, space="PSUM"))
        yp_pool = ctx.enter_context(tc.tile_pool(name="yp", bufs=1, space="PSUM"))
        rid_pool = ctx.enter_context(tc.tile_pool(name="rid", bufs=1, space="PSUM"))

        xT = persist.tile([128, 4, R], BF16, name="xT")
        wq_sb = persist.tile([128, 4, C], BF16, name="wq_sb")
        wk_sb = persist.tile([128, 4, C], BF16, name="wk_sb")
        wv_sb = persist.tile([128, 4, C], BF16, name="wv_sb")
        wo_sb = persist.tile([128, 4, C], BF16, name="wo_sb")
        yT = persist.tile([128, 4, R], BF16, name="yT")
        vA = [persist.tile([128, 6, H, HD + 1], BF16, name=f"vA{b}", tag=f"vA{b}")
              for b in range(BPC)]
        # qka[b][cc]: [128, 2(q/k), 2(val/aug), L] fp8 for the DR S matmul
        qka = [[persist.tile([128, 2, 2, L], F8, name=f"qka{b}{cc}",
                             tag=f"qka{b}{cc}") for cc in range(4)]
               for b in range(BPC)]

        # fine-grained input DMAs: b0 x-chunks + Wq/Wk first so the first
        # projection (and first exp) starts as early as possible; issues
        # spread across the three HWDGE-capable sequencers (SP/ACT/DVE)
        xt_r = xt_d.rearrange("(c p) (b r) -> p c b r", p=128, b=BPC)
        xT_v = xT.rearrange("p c (b r) -> p c b r", b=BPC)
        nc.sync.dma_start(out=xT_v[:, 0, 0], in_=xt_r[:, 0, 0])
        nc.scalar.dma_start(out=wq_sb, in_=wq_d.rearrange("(c p) o -> p c o", p=128))
        nc.scalar.dma_start(out=wk_sb, in_=wk_d.rearrange("(c p) o -> p c o", p=128))
        nc.sync.dma_start(out=xT_v[:, 1, 0], in_=xt_r[:, 1, 0])
        nc.sync.dma_start(out=xT_v[:, 2, 0], in_=xt_r[:, 2, 0])
        nc.sync.dma_start(out=xT_v[:, 3, 0], in_=xt_r[:, 3, 0])
        nc.scalar.dma_start(out=qka[0][0][:, 0, 1, :], in_=oh_d)
        nc.scalar.dma_start(out=qka[0][0][:, 1, 1, :], in_=kb_d[0])
        nc.gpsimd.dma_start(out=wv_sb, in_=wv_d.rearrange("(c p) o -> p c o", p=128))
        nc.sync.dma_start(out=wo_sb, in_=wo_d.rearrange("(c p) o -> p c o", p=128))
        nc.sync.dma_start(out=xT_v[:, :, 1], in_=xt_r[:, :, 1])
        for b in range(BPC):
            for cc in range(4):
                if b == 0 and cc == 0:
                    continue
                dma = nc.sync.dma_start if (b == 0 and cc == 1) else \
                    nc.gpsimd.dma_start
                dma(out=qka[b][cc][:, 0, 1, :], in_=oh_d)
                dma(out=qka[b][cc][:, 1, 1, :], in_=kb_d[cc])
            nc.vector.memset(vA[b][:, :, :, 0:1], 1.0)

        def proj_qk1(b, cc, qk):
            # one projection (q or k), two 512/256-col rider pieces
            w_sb = wq_sb if qk == 0 else wk_sb
            for p0, p1 in ((0, 512), (512, L)):
                ps = rid_pool.tile([128, 512], F32, name="rid", tag="rid")
                for ci in range(4):
                    nc.tensor.matmul(
                        ps[:, 0:p1 - p0],
                        lhsT=w_sb[:, ci, cc * 128:(cc + 1) * 128],
                        rhs=xT[:, ci, b * L + p0:b * L + p1],
                        start=(ci == 0), stop=(ci == 3))
                nc.vector.tensor_copy(out=qka[b][cc][:, qk, 0, p0:p1],
                                      in_=ps[:, 0:p1 - p0])

        def proj_qk(b, cc):
            proj_qk1(b, cc, 0)
            proj_qk1(b, cc, 1)

        def proj_qk_sp(b, cc, qk, pool, plen):
            # pre-loop variant: full 768-col projection in an sp-pool tile
            w_sb = wq_sb if qk == 0 else wk_sb
            tag = "sp0" if plen == SP0_LEN else "sp12"
            ps = pool.tile([128, plen], F32, name=tag, tag=tag)
            for p0, p1 in _bank_pieces(0, L):
                for ci in range(4):
                    nc.tensor.matmul(
                        ps[:, p0:p1],
                        lhsT=w_sb[:, ci, cc * 128:(cc + 1) * 128],
                        rhs=xT[:, ci, b * L + p0:b * L + p1],
                        start=(ci == 0), stop=(ci == 3))
            nc.vector.tensor_copy(out=qka[b][cc][:, qk, 0, :], in_=ps[:, 0:L])

        def proj_v2_sp(b, lp):
            # pre-loop variant: two v row-chunks in one sp12 tile
            ps = sp12_pool.tile([128, SP12_LEN], F32, name="sp12", tag="sp12")
            for half in range(2):
                l = 2 * lp + half
                for ci in range(4):
                    nc.tensor.matmul(
                        ps[:, half * C:(half + 1) * C],
                        lhsT=xT[:, ci, b * L + l * 128:b * L + (l + 1) * 128],
                        rhs=wv_sb[:, ci, :],
                        start=(ci == 0), stop=(ci == 3))
            nc.vector.tensor_copy(
                out=vA[b][:, 2 * lp:2 * lp + 2, :, 1:HD + 1],
                in_=ps[:, 0:2 * C].rearrange("p (a h d) -> p a h d", a=2, h=H))

        def proj_v(b, l):
            ps = rid_pool.tile([128, 512], F32, name="rid", tag="rid")
            for ci in range(4):
                nc.tensor.matmul(
                    ps,
                    lhsT=xT[:, ci, b * L + l * 128:b * L + (l + 1) * 128],
                    rhs=wv_sb[:, ci, :],
                    start=(ci == 0), stop=(ci == 3))
            nc.vector.tensor_copy(
                out=vA[b][:, l, :, 1:HD + 1],
                in_=ps.rearrange("p (h d) -> p h d", h=H))

        def final(b, ic, pool, plen):
            tag = {SP0_LEN: "sp0", SP12_LEN: "sp12", 512: "rid"}[plen]
            ps = pool.tile([128, plen], F32, name=tag, tag=tag)
            for cc in range(4):
                nc.tensor.matmul(
                    ps[:, 0:C],
                    lhsT=yT[:, cc, b * L + ic * 128:b * L + (ic + 1) * 128],
                    rhs=wo_sb[:, cc, :],
                    start=(cc == 0), stop=(cc == 3))
            fo = fo_pool.tile([128, C], F32, name="fo", tag="fo")
            nc.vector.tensor_copy(out=fo, in_=ps[:, 0:C])
            nc.sync.dma_start(
                out=out_d[b * L + ic * 128:b * L + (ic + 1) * 128, :], in_=fo)

        yn_cur = [None]
        live = {}

        def unit_S(b, h):
            cc, par = divmod(h, 2)
            base = par * 64
            qs = qka[b][cc][base:base + 64, 0, :, :]
            ks = qka[b][cc][base:base + 64, 1, :, :]
            s0 = sp0_pool.tile([128, SP0_LEN], F32, name="sp0", tag="sp0")
            s12 = sp12_pool.tile([128, SP12_LEN], F32, name="sp12", tag="sp12")
            for jc in range(6):
                st = s0 if jc < 2 else s12
                off, i0 = S_OFF[jc], S_I0[jc]
                for p0, p1 in _bank_pieces(off, off + (L - i0)):
                    nc.tensor.matmul(
                        st[:, p0:p1],
                        lhsT=ks[:, :, jc * 128:(jc + 1) * 128],
                        rhs=qs[:, :, i0 + (p0 - off):i0 + (p1 - off)],
                        start=True, stop=True, perf_mode=DR)
            live[(b, h)] = (s0, s12)

        def unit_exp(b, h):
            s0, s12 = live.pop((b, h))
            p0t = pts0_pool.tile([128, SP0_LEN], BF16, name="pts0", tag="pts0")
            p12t = pts12_pool.tile([128, SP12_LEN], BF16, name="pts12", tag="pts12")
            nc.scalar.activation(out=p0t, in_=s0, func=EXP,
                                 bias=0.0, scale=ESCALE)
            nc.scalar.activation(out=p12t, in_=s12, func=EXP,
                                 bias=0.0, scale=ESCALE)
            live[(b, h, "p")] = (p0t, p12t)

        def unit_pv(b, h):
            cc, par = divmod(h, 2)
            base = par * 64
            p0t, p12t = live.pop((b, h, "p"))
            yp = yp_pool.tile([128, 6, HD + 1], F32, name="yp", tag="yp")
            for ic in range(6):
                for jc in range(ic + 1):
                    pt = p0t if jc < 2 else p12t
                    col = S_OFF[jc] + ic * 128 - S_I0[jc]
                    nc.tensor.matmul(
                        yp[:, ic, :],
                        lhsT=pt[:, col:col + 128],
                        rhs=vA[b][:, jc, h, :],
                        start=(jc == 0), stop=(jc == ic))
            # stage yp out to SBUF immediately so the single yp bank frees
            # fast (PV(n+1) -> copy(n) instead of the full normalize chain)
            ys = ys_pool.tile([128, 6, HD + 1], F32, name="ys", tag="ys")
            nc.vector.tensor_copy(out=ys, in_=yp)
            rc = rc_pool.tile([128, 6], F32, name="rc", tag="rc")
            nc.vector.reciprocal(out=rc, in_=ys[:, :, 0])
            if par == 0:
                yn_cur[0] = yn_pool.tile([128, 6, 128], BF16, name="yn", tag="yn")
            yn = yn_cur[0]
            nc.vector.tensor_tensor(
                yn[:, :, base:base + HD], ys[:, :, 1:HD + 1],
                rc[:, :, None].broadcast_to([128, 6, HD]), MUL)
            if par == 1:
                for ic in range(6):
                    nc.sync.dma_start_transpose(
                        out=yT[:, cc, b * L + ic * 128:b * L + (ic + 1) * 128],
                        in_=yn[:, ic, :])

        # ---- schedule: software-pipelined units with riders ----
        seq = [(0, h) for h in range(8)] + [(1, h) for h in range(8)]

        def fin0(ic):
            return lambda: final(0, ic, rid_pool, 512)

        riders = {
            0: [lambda: proj_qk(0, 2)],
            2: [lambda: proj_qk(0, 3)],
            3: [lambda: proj_v(1, 0), lambda: proj_v(1, 1)],
            4: [lambda: proj_qk(1, 0)],
            5: [lambda: proj_v(1, 2), lambda: proj_v(1, 3)],
            6: [lambda: proj_qk(1, 1)],
            7: [lambda: proj_v(1, 4), lambda: proj_v(1, 5)],
            8: [lambda: proj_qk(1, 2)],
            9: [fin0(0)],
            10: [lambda: proj_qk(1, 3)],
            11: [fin0(1)], 12: [fin0(2)], 13: [fin0(3)],
            14: [fin0(4)], 15: [fin0(5)],
        }
        proj_qk_sp(0, 0, 0, sp0_pool, SP0_LEN)
        proj_qk_sp(0, 0, 1, sp12_pool, SP12_LEN)
        proj_v(0, 0)
        proj_qk_sp(0, 1, 0, sp0_pool, SP0_LEN)
        proj_v(0, 1)
        proj_qk_sp(0, 1, 1, sp12_pool, SP12_LEN)
        for l in range(2, 6):
            proj_v(0, l)
        unit_S(*seq[0])
        for n, (b, h) in enumerate(seq):
            unit_exp(b, h)
            if n + 1 < len(seq):
                unit_S(*seq[n + 1])
            unit_pv(b, h)
            for r in riders.get(n, []):
                r()
        # tail: junk burst keeps the PE p-state hot through the last
        # normalize/XBAR wait, then finals on a 3-pool rotation with
        # ACT-side copies (DVE is busy with the last units' normalize)
        for w in range(5):
            jp = yp_pool.tile([128, 6, HD + 1], F32, name="yp", tag="yp")
            for _ in range(8):
                nc.tensor.matmul(jp.rearrange("p a b -> p (a b)"),
                                 lhsT=yT[:, 0, 0:128], rhs=yT[:, 0, 0:390],
                                 start=True, stop=True)
        for ic in range(6):
            pool, plen = [(sp0_pool, SP0_LEN), (sp12_pool, SP12_LEN),
                          (rid_pool, 512)][ic % 3]
            tag = {SP0_LEN: "sp0", SP12_LEN: "sp12", 512: "rid"}[plen]
            ps = pool.tile([128, plen], F32, name=tag, tag=tag)
            for cc in range(4):
                nc.tensor.matmul(
                    ps[:, 0:C],
                    lhsT=yT[:, cc, L + ic * 128:L + (ic + 1) * 128],
                    rhs=wo_sb[:, cc, :],
                    start=(cc == 0), stop=(cc == 3))
            fo = fo_pool.tile([128, C], F32, name="fo", tag="fo")
            nc.scalar.copy(out=fo, in_=ps[:, 0:C])
            nc.sync.dma_start(
                out=out_d[L + ic * 128:L + (ic + 1) * 128, :], in_=fo)


def host_tables(rel_pos_bias, particle_rel_pos_bias):
    """onehot [128, L] (2 identical halves) and kbias8 pairs [4, 128, L]."""
    bt = np.asarray(rel_pos_bias, np.float32)
    bp = np.asarray(particle_rel_pos_bias, np.float32)
    i = np.arange(L)
    t1, n1 = i // NP, i % NP
    oh64 = np.zeros((64, L), np.float32)
    oh64[t1, i] = 1.0
    oh64[T + n1, i] = 1.0
    oh = np.concatenate([oh64, oh64], axis=0)

    j = np.arange(L)
    t2, n2 = j // NP, j % NP
    tt = np.arange(T)[:, None]
    ktop = 8.0 * bt[(t2[None, :] - tt) + (T - 1)]             # [T, L, H]
    ktop = np.transpose(ktop, (2, 0, 1))
    ktop = ktop + np.where(t2[None, :] > tt, MASK, 0.0)[None]
    nn = np.arange(NP)[:, None]
    kbot = 8.0 * bp[(n2[None, :] - nn) + (NP - 1)]
    kbot = np.transpose(kbot, (2, 0, 1))
    kb_h = np.concatenate([ktop, kbot], axis=1)               # [H, 64, L]
    kb = np.stack([np.concatenate([kb_h[2 * cc], kb_h[2 * cc + 1]], axis=0)
                   for cc in range(4)])                       # [4, 128, L]
    kb = np.clip(kb, -240.0, 240.0)
    return oh.astype(FP8), np.ascontiguousarray(kb.astype(FP8))


def make_in_maps(x, Wq, Wk, Wv, Wo, rel_pos_bias, particle_rel_pos_bias):
    x = np.asarray(x, np.float32)
    xt = np.ascontiguousarray(np.transpose(x, (0, 3, 2, 1))).reshape(B_TOT, C, L)
    xt16 = xt.astype(BF)
    ws = [np.ascontiguousarray(np.asarray(w, np.float32).astype(BF))
          for w in (Wq, Wk, Wv, Wo)]
    oh, kb = host_tables(rel_pos_bias, particle_rel_pos_bias)
    in_maps = []
    for c in range(NCORES):
        xc = np.ascontiguousarray(
            np.concatenate([xt16[2 * c], xt16[2 * c + 1]], axis=1))
        in_maps.append({"xt": xc, "wq": ws[0], "wk": ws[1], "wv": ws[2],
                        "wo": ws[3], "oh": oh, "kb": kb})
    return in_maps


_NC_CACHE = None


def _get_nc():
    global _NC_CACHE
    if _NC_CACHE is None:
        _NC_CACHE = build_nc()
    return _NC_CACHE


def unshard_out(res):
    outs = []
    for c in range(NCORES):
        o = res.results[c]["out"].astype(np.float32)
        o = o.reshape(BPC, T, NP, C).transpose(0, 2, 1, 3)
        outs.append(o)
    return np.ascontiguousarray(np.concatenate(outs, axis=0))


def kernel(x, Wq, Wk, Wv, Wo, rel_pos_bias, particle_rel_pos_bias):
    in_maps = make_in_maps(x, Wq, Wk, Wv, Wo, rel_pos_bias, particle_rel_pos_bias)
    res = bass_utils.run_bass_kernel_spmd(
        _get_nc(), in_maps, core_ids=list(range(NCORES)))
    return unshard_out(res)
